# revision 29
# baseline (speedup 1.0000x reference)
"""CRPS loss kernel for Trainium2 (8 NeuronCores, batch-parallel).

Math per grid point (N=32 members x_i, target y, lat weight w_h):
  CRPS = (1/N) sum_i |x_i - y| - (1/N^2) sum_{i<j} (x_(j) - x_(i))
Everything is positively homogeneous in w_h, so the host pre-multiplies
inputs by w_h (f64) before the bf16 cast and the device computes plain
global sums.  With
  |a-b| = 2 max(a,b) - a - b
  sum_{i<j}(x_(j)-x_(i)) = 2 sum_{i<j} max(x_i,x_j) - (N-1) sum_i x_i
  max(a,b) = (a + b + |a-b|) / 2
the nonlinear device work is sums of pairwise maxes / absolute
differences; all linear sums go to the host in f64.

Layout per core: points (2 batches x 121 lat x 240 lon = 58,080, padded
to 128*454) on the partition axis, 33 "members" (32 ensemble + target)
in a free axis: X [128, 33, 454] bf16.  The 528 member pairs (incl. 32
y-pairs) split three ways:
  - DVE tensor_max (bf16 2x mode): the 32 y-pairs as one broadcast max,
    a 17-row partial of shift 5, and shifts 6..31 as 13 uniform 27-row
    slots (shift pairs (a,b), a+b=37).  400 pair-rows total.
  - PE difference matmuls: 128 pairs (shifts 1..4 plus 10 rows of
    shift 5) as one +/-1 stationary matrix against a transposed copy
    X2 [33, points] streamed from DRAM; Act abs+accumulates the f32
    diffs from PSUM (1536-col reads).
  - Accumulation of DVE slots: PE identity-matmul chains into PSUM
    (bank A = x-pairs, bank B = y-pairs) for 12 units; Act directly
    Copy+accum_outs the first two 27-row slots to offload PE.
Output per core: [128, 5] f32 partial sums; host combines in f64.
"""

import numpy as np
import ml_dtypes

import concourse.bass as bass
import concourse.mybir as mybir
from concourse.bass_utils import run_bass_kernel_spmd

H, W, B, N = 121, 240, 16, 32
N_CORES = 8
B_LOC = B // N_CORES

PTS = B_LOC * H * W          # 58,080 real points per core
PP = 454                     # free-dim columns per partition
NPTS = 128 * PP              # 58,112 padded points
M = N + 1                    # 32 ensemble members + target

# --- pair assignment ---------------------------------------------------
# PE-diff pairs: shifts 1..4 (118 pairs) + first 10 rows of shift 5.
PE_PAIRS = [(i, i + d) for d in range(1, 5) for i in range(N - d)] + [
    (i, i + 5) for i in range(10)
]
assert len(PE_PAIRS) == 128
S5_LO, S5_ROWS = 10, 17      # shift-5 rows 10..26 on DVE
DVE_PAIRS = [(6, 31), (7, 30), (8, 29), (9, 28), (10, 27), (11, 26),
             (12, 25), (13, 24), (14, 23), (15, 22), (16, 21), (17, 20),
             (18, 19)]
SLOT_ROWS = 27               # (32-a)+(32-b) with a+b=37
N_ACT_SLOTS = 1              # first DVE slot accumulated by Act

CH = 512                     # psum chunk columns
GCOLS = 3 * CH               # 1536-col diff groups (3 banks)
NGRP = (NPTS + GCOLS - 1) // GCOLS       # 38 diff groups per iter
XCH_G = 3                    # x2 dma chunk = 3 groups
NXCH = (NGRP + XCH_G - 1) // XCH_G       # 13 x2 chunks per iter
OUT_COLS = 5                 # pA, pB, D, act0, s5 sums
SUB_ROWS = 3                 # act slot sub-accum rows (27 = 9 x 3)
SUBS = SLOT_ROWS // SUB_ROWS             # 9
SUBS5 = (S5_ROWS + SUB_ROWS - 1) // SUB_ROWS   # 6 (last is 2 rows)


F32 = mybir.dt.float32
BF16 = mybir.dt.bfloat16
ALU = mybir.AluOpType
AFT = mybir.ActivationFunctionType

_NC_CACHE = {}


def _grp_cols(g):
    lo = g * GCOLS
    return lo, min(NPTS, lo + GCOLS)


def build_nc(repeat=1, detect_races=True):
    key = (repeat, detect_races)
    if key in _NC_CACHE:
        return _NC_CACHE[key]
    nc = bass.Bass(detect_race_conditions=detect_races)
    x_in = nc.declare_dram_parameter("x", [128, M * PP], BF16, isOutput=False)
    x2_in = nc.declare_dram_parameter("x2", [M, NPTS], BF16, isOutput=False)
    m_in = nc.declare_dram_parameter("mpairs", [M, 128], BF16, isOutput=False)
    i_in = nc.declare_dram_parameter("ident", [128, 128], BF16, isOutput=False)
    o_out = nc.declare_dram_parameter("o", [128, OUT_COLS], F32, isOutput=True)

    SLOT_FLAT = SLOT_ROWS * PP           # 12,258
    Y_FLAT = N * PP                      # 14,528
    S5_FLAT = S5_ROWS * PP               # 7,718
    NCH_S = (SLOT_FLAT + CH - 1) // CH   # 24
    NCH_Y = (Y_FLAT + CH - 1) // CH      # 29
    NCH_5 = (S5_FLAT + CH - 1) // CH     # 16
    NSLOT = len(DVE_PAIRS)               # 13
    XCOLS = XCH_G * GCOLS                # 4608 cols per x2 chunk

    # ---------- static schedules ----------
    # DVE production order (unit name, ring buffer id)
    dve_order = ["act0", "y", "s5"] + [f"slot{k}" for k in range(1, NSLOT)]
    VPER = len(dve_order)                # 15
    v_pos = {u: i + 1 for i, u in enumerate(dve_order)}  # sem value when done

    # PE order: interleave diff work (by x2 chunk) with accumulation so PE
    # has work before DVE's first PE-slot lands and chunks stay 2-ring.
    pe_order = []
    pe_order += [("chunk", 0), ("chunk", 1)]
    pe_order += [("y", None), ("chunk", 2)]
    ci = 3
    for k in range(1, NSLOT):
        pe_order.append(("slot", k))
        if ci < NXCH - 1 and k <= 9:
            pe_order.append(("chunk", ci))
            ci += 1
    while ci < NXCH:
        pe_order.append(("chunk", ci))
        ci += 1
    # p_sem increment schedule: +1 per diff GROUP, per y/s5/slot unit.
    p_pos = {}       # unit -> p_sem value when its last inc fires
    g_pos = {}       # diff group -> p_sem value when done
    cnt = 0
    for kind, arg in pe_order:
        if kind == "chunk":
            for g in range(arg * XCH_G, min(NGRP, (arg + 1) * XCH_G)):
                cnt += 1
                g_pos[g] = cnt
            p_pos[("chunk", arg)] = cnt
        else:
            cnt += 1
            p_pos[(kind, arg)] = cnt
    PPER = cnt                           # p_sem incs per iter (38+13=51)

    from contextlib import ExitStack

    with ExitStack() as ctx:
        xt = ctx.enter_context(nc.sbuf_tensor([128, M, PP], BF16))
        x2t = ctx.enter_context(nc.sbuf_tensor([M, 2, XCOLS], BF16))
        mp = ctx.enter_context(nc.sbuf_tensor([M, 128], BF16))
        ident = ctx.enter_context(nc.sbuf_tensor([128, 128], BF16))
        sa = ctx.enter_context(nc.sbuf_tensor([128, SLOT_ROWS, PP], BF16))
        sb = ctx.enter_context(nc.sbuf_tensor([128, SLOT_ROWS, PP], BF16))
        sc = ctx.enter_context(nc.sbuf_tensor([128, SLOT_ROWS, PP], BF16))
        sact = ctx.enter_context(nc.sbuf_tensor([128, SLOT_ROWS, PP], BF16))
        ys = ctx.enter_context(nc.sbuf_tensor([128, N, PP], BF16))
        s5t = ctx.enter_context(nc.sbuf_tensor([128, S5_ROWS, PP], BF16))
        dump_g0 = ctx.enter_context(nc.sbuf_tensor([128, GCOLS], BF16))
        dump_g1 = ctx.enter_context(nc.sbuf_tensor([128, GCOLS], BF16))
        acc_d = ctx.enter_context(nc.sbuf_tensor([128, NGRP], F32))
        acc_a = ctx.enter_context(nc.sbuf_tensor([128, SUBS], F32))
        fd_d = ctx.enter_context(nc.sbuf_tensor([128, NGRP], F32))
        fd_a = ctx.enter_context(nc.sbuf_tensor([128, SUBS], F32))
        fd_s5 = ctx.enter_context(nc.sbuf_tensor([128, SUBS5], F32))
        acc_s5 = ctx.enter_context(nc.sbuf_tensor([128, SUBS5], F32))
        ot = ctx.enter_context(nc.sbuf_tensor([128, OUT_COLS], F32))
        pA = ctx.enter_context(nc.psum_tensor([128, CH], F32))
        pB = ctx.enter_context(nc.psum_tensor([128, CH], F32))
        pd0 = ctx.enter_context(nc.psum_tensor([128, GCOLS], F32))
        pd1 = ctx.enter_context(nc.psum_tensor([128, GCOLS], F32))
        dma_sem = ctx.enter_context(nc.semaphore())
        x2_sem = ctx.enter_context(nc.semaphore())  # x2 chunk dma arrivals
        v_sem = ctx.enter_context(nc.semaphore())
        p_sem = ctx.enter_context(nc.semaphore())
        r_sem = ctx.enter_context(nc.semaphore())   # ACT diff-reads
        a_sem = ctx.enter_context(nc.semaphore())   # ACT act0-slot accums
        s5_sem = ctx.enter_context(nc.semaphore())  # ACT s5 accums
        f_sem = ctx.enter_context(nc.semaphore())   # ACT pA/pB finals done
        s_sem = ctx.enter_context(nc.semaphore())   # ACT finals
        block = ctx.enter_context(nc.Block())
        xv = xt[:]
        ring = [sa, sb, sc]
        ring_flat = [sa[:].rearrange("p r w -> p (r w)"),
                     sb[:].rearrange("p r w -> p (r w)"),
                     sc[:].rearrange("p r w -> p (r w)")]
        ys_flat = ys[:].rearrange("p r w -> p (r w)")
        s5_flat = s5t[:].rearrange("p r w -> p (r w)")
        pd = [pd0, pd1]

        @block.sync
        def _(sync):
            sync.dma_start(
                out=xt[:],
                in_=x_in[:].rearrange("p (m w) -> p m w", m=M, w=PP),
            ).then_inc(dma_sem, 16)
            sync.dma_start(out=ident[:], in_=i_in[:]).then_inc(dma_sem, 16)
            sync.dma_start(out=mp[:], in_=m_in[:]).then_inc(dma_sem, 16)
            for it in range(repeat):
                for c in range(NXCH):
                    ci_abs = it * NXCH + c
                    if ci_abs >= 1:
                        # anchor chunk completion order for the race detector
                        sync.wait_ge(x2_sem, 16 * ci_abs)
                    if ci_abs >= 2:
                        # buffer reuse: PE done with chunk ci_abs-2
                        pit, pc = divmod(ci_abs - 2, NXCH)
                        sync.wait_ge(p_sem, pit * PPER + p_pos[("chunk", pc)])
                    lo = c * XCOLS
                    hi = min(NPTS, lo + XCOLS)
                    sync.dma_start(
                        out=x2t[:, ci_abs % 2, : hi - lo],
                        in_=x2_in[:, lo:hi],
                    ).then_inc(x2_sem, 16)
            sync.wait_ge(s_sem, repeat)
            sync.dma_start(out=o_out[:], in_=ot[:]).then_inc(dma_sem, 16)

        @block.vector
        def _(vector):
            vector.wait_ge(dma_sem, 48)
            ybc = xt[:, N : N + 1, :].broadcast_to((128, N, PP))
            for it in range(repeat):
                vb = VPER * it
                for ui, unit in enumerate(dve_order):
                    # Ring users in order: act0 (=k 0), slot1..slot12; the
                    # buffer of user u is reused by user u+3.  act0 is
                    # consumed by ACT (a_sem); PE slots by p_sem.
                    if unit == "y":
                        if it > 0:
                            vector.wait_ge(p_sem, (it - 1) * PPER + p_pos[("y", None)])
                        nc.vector.tensor_max(
                            ys[:], xt[:, 0:N, :], ybc
                        ).then_inc(v_sem, 1)
                        continue
                    if unit == "s5":
                        if it > 0:
                            vector.wait_ge(s5_sem, it)  # ACT s5-accum done
                        nc.vector.tensor_max(
                            s5t[:],
                            xv[:, S5_LO + 5 : S5_LO + 5 + S5_ROWS, :],
                            xv[:, S5_LO : S5_LO + S5_ROWS, :],
                        ).then_inc(v_sem, 1)
                        continue
                    if unit == "act0":
                        if it > 0:
                            vector.wait_ge(a_sem, it)  # ACT act0-accum done
                        da, db = DVE_PAIRS[0]
                        ra = N - da
                        nc.vector.tensor_max(
                            sact[:, :ra, :], xv[:, da:N, :], xv[:, : N - da, :]
                        )
                        nc.vector.tensor_max(
                            sact[:, ra:SLOT_ROWS, :],
                            xv[:, db:N, :],
                            xv[:, : N - db, :],
                        ).then_inc(v_sem, 1)
                        continue
                    k = int(unit[4:])
                    # ring[(k-1) % 3]: previous occupant is slot k-3, or
                    # slot k+9 of the previous iteration for k in {1,2,3}
                    if k >= 4:
                        vector.wait_ge(p_sem, it * PPER + p_pos[("slot", k - 3)])
                    elif it > 0:
                        vector.wait_ge(
                            p_sem, (it - 1) * PPER + p_pos[("slot", k + 9)]
                        )
                    slot = ring[(k - 1) % 3]
                    da, db = DVE_PAIRS[k]
                    ra = N - da
                    nc.vector.tensor_max(
                        slot[:, :ra, :], xv[:, da:N, :], xv[:, : N - da, :]
                    )
                    nc.vector.tensor_max(
                        slot[:, ra:SLOT_ROWS, :],
                        xv[:, db:N, :],
                        xv[:, : N - db, :],
                    ).then_inc(v_sem, 1)

        @block.tensor
        def _(tensor):
            tensor.wait_ge(dma_sem, 48)

            def accum_chain(psum, flat, nch, flat_len, start, stop):
                last = None
                for c in range(nch):
                    lo = c * CH
                    hi = min(flat_len, lo + CH)
                    last = tensor.matmul(
                        psum[:, : hi - lo],
                        ident[:],
                        flat[:, lo:hi],
                        start=(start and c == 0),
                        stop=(stop and c == nch - 1),
                    )
                return last

            for it in range(repeat):
                vb = VPER * it
                if it > 0:
                    tensor.wait_ge(f_sem, it)  # ACT read pA/pB of prev iter
                for kind, arg in pe_order:
                    if kind == "chunk":
                        c = arg
                        ci_abs = it * NXCH + c
                        tensor.wait_ge(x2_sem, 16 * (ci_abs + 1))
                        xbuf = x2t[:, ci_abs % 2, :]
                        for g in range(c * XCH_G, min(NGRP, (c + 1) * XCH_G)):
                            gi_abs = it * NGRP + g
                            if gi_abs >= 2:
                                tensor.wait_ge(r_sem, gi_abs - 1)
                            lo, hi = _grp_cols(g)
                            xlo = lo - c * XCOLS
                            last = None
                            for s in range(0, hi - lo, CH):
                                n = min(CH, hi - lo - s)
                                last = tensor.matmul(
                                    pd[gi_abs % 2][:, s : s + n],
                                    mp[:],
                                    xbuf[:, xlo + s : xlo + s + n],
                                    start=True,
                                    stop=True,
                                )
                            last.then_inc(p_sem, 1)
                    elif kind == "y":
                        tensor.wait_ge(v_sem, vb + v_pos["y"])
                        accum_chain(pB, ys_flat, NCH_Y, Y_FLAT, True, True).then_inc(
                            p_sem, 1
                        )
                    else:  # slot k
                        k = arg
                        tensor.wait_ge(v_sem, vb + v_pos[f"slot{k}"])
                        last = None
                        for c in range(NCH_S):
                            lo = c * CH
                            hi = min(SLOT_FLAT, lo + CH)
                            last = tensor.matmul(
                                pA[:, : hi - lo],
                                ident[:],
                                ring_flat[(k - 1) % 3][:, lo:hi],
                                start=(k == 1 and c == 0),
                                stop=(k == NSLOT - 1 and c == NCH_S - 1),
                            )
                        last.then_inc(p_sem, 1)

        @block.scalar
        def _(scalar):
            # Per-iteration ACT order: diff reads with act0/s5 sub-accums
            # interleaved (pauses <= ~1.5us each so the 2-deep PE<->ACT psum
            # ring never starves), pA/pB finals slotted before the last two
            # reads (they only need slot12, freeing next iter's PE via
            # f_sem), then the accumulator roll-ups.
            ops = []
            gi = 0
            for s in range(SUBS):            # act0: 9 subs against reads 0-8
                ops.append(("read", gi)); gi += 1
                ops.append(("suba", s))
            for s in range(SUBS5):           # s5: 6 subs against reads 9-14
                ops.append(("read", gi)); gi += 1
                ops.append(("sub5", s))
            while gi < NGRP - 2:
                ops.append(("read", gi)); gi += 1
            ops.append(("finAB", None))
            while gi < NGRP:
                ops.append(("read", gi)); gi += 1
            ops.append(("finD", None))

            dump_g = [dump_g0, dump_g1]
            for it in range(repeat):
                vb = VPER * it
                for kind, arg in ops:
                    if kind == "read":
                        g = arg
                        gi_abs = it * NGRP + g
                        if g == 0 and it > 0:
                            scalar.wait_ge(s_sem, it)  # prev finD read acc_d
                        scalar.wait_ge(p_sem, it * PPER + g_pos[g])
                        lo, hi = _grp_cols(g)
                        nc.scalar.activation(
                            dump_g[g % 2][:, : hi - lo],
                            pd[gi_abs % 2][:, : hi - lo],
                            AFT.Abs,
                            accum_out=acc_d[:, g : g + 1],
                        ).then_inc(r_sem, 1)
                    elif kind == "suba":
                        s = arg
                        if s == 0:
                            scalar.wait_ge(v_sem, vb + v_pos["act0"])
                        r0 = s * SUB_ROWS
                        sl = sact[:, r0 : r0 + SUB_ROWS, :].rearrange(
                            "p r w -> p (r w)"
                        )
                        act = nc.scalar.activation(
                            sl, sl, AFT.Copy, accum_out=acc_a[:, s : s + 1]
                        )
                        if s == SUBS - 1:
                            act.then_inc(a_sem, 1)
                    elif kind == "sub5":
                        s = arg
                        if s == 0:
                            scalar.wait_ge(v_sem, vb + v_pos["s5"])
                        r0 = s * SUB_ROWS
                        rows = min(SUB_ROWS, S5_ROWS - r0)
                        sl = s5t[:, r0 : r0 + rows, :].rearrange("p r w -> p (r w)")
                        act = nc.scalar.activation(
                            sl, sl, AFT.Copy, accum_out=acc_s5[:, s : s + 1]
                        )
                        if s == SUBS5 - 1:
                            act.then_inc(s5_sem, 1)
                    elif kind == "finAB":
                        scalar.wait_ge(p_sem, it * PPER + p_pos[("slot", NSLOT - 1)])
                        nc.scalar.activation(
                            pA[:], pA[:], AFT.Copy, accum_out=ot[:, 0:1]
                        )
                        nc.scalar.activation(
                            pB[:], pB[:], AFT.Copy, accum_out=ot[:, 1:2]
                        ).then_inc(f_sem, 1)
                    else:  # finD
                        scalar.wait_ge(r_sem, (it + 1) * NGRP)
                        scalar.wait_ge(a_sem, it + 1)
                        scalar.wait_ge(s5_sem, it + 1)
                        nc.scalar.activation(
                            fd_d[:], acc_d[:], AFT.Copy, accum_out=ot[:, 2:3]
                        )
                        nc.scalar.activation(
                            fd_a[:], acc_a[:], AFT.Copy, accum_out=ot[:, 3:4]
                        )
                        nc.scalar.activation(
                            fd_s5[:], acc_s5[:], AFT.Copy, accum_out=ot[:, 4:5]
                        ).then_inc(s_sem, 1)

    _NC_CACHE[key] = nc
    return nc


def _lat_weights_f64():
    lats = np.arange(90.0, -91.5, -1.5)  # [121]
    w = np.cos(np.deg2rad(lats))
    return H * (w / np.sum(w))


def _prep_inputs(predictions, targets):
    """Full f32 -> per-core packed bf16 maps (pre-weighted, padded)."""
    w = _lat_weights_f64()
    p = np.asarray(predictions, dtype=np.float64) * w[None, None, :, None]
    t = np.asarray(targets, dtype=np.float64) * w[None, :, None]
    ident = np.eye(128).astype(ml_dtypes.bfloat16)
    mpairs = np.zeros((M, 128), ml_dtypes.bfloat16)
    for c, (i, j) in enumerate(PE_PAIRS):
        mpairs[i, c] = 1
        mpairs[j, c] = -1
    in_maps = []
    for c in range(N_CORES):
        pc = p[B_LOC * c : B_LOC * (c + 1)]          # [2,32,121,240]
        tc = t[B_LOC * c : B_LOC * (c + 1)]          # [2,121,240]
        arr = np.zeros((NPTS, M), np.float64)
        arr[:PTS, :N] = pc.transpose(0, 2, 3, 1).reshape(PTS, N)
        arr[:PTS, N] = tc.reshape(PTS)
        xw = arr.astype(ml_dtypes.bfloat16)          # [58112, 33]
        xc = np.ascontiguousarray(
            xw.reshape(128, PP, M).transpose(0, 2, 1)
        ).reshape(128, M * PP)
        x2 = np.ascontiguousarray(xw.T)              # [33, 58112]
        in_maps.append({"x": xc, "x2": x2, "mpairs": mpairs, "ident": ident})
    return in_maps


def _member_sums(in_maps):
    """f64 per-member weighted sums from the exact bf16 device values."""
    s_m = np.zeros(N, np.float64)
    sy = 0.0
    for m in in_maps:
        x2 = m["x2"].astype(np.float64)              # [33, NPTS]
        s_m += x2[:N].sum(axis=1)
        sy += x2[N].sum()
    return s_m, sy


def _combine(outs, in_maps):
    """outs: list of [128, OUT_COLS] f32 -> scalar f32 (host math in f64)."""
    s_m, sy = _member_sums(in_maps)
    A = B_ = D = 0.0
    for o in outs:
        o = np.asarray(o, dtype=np.float64)
        A += o[:, 0].sum() + o[:, 3].sum() + o[:, 4].sum()
        B_ += o[:, 1].sum()
        D += o[:, 2].sum()
    degP = np.zeros(N, np.float64)
    for (i, j) in PE_PAIRS:
        degP[i] += 1
        degP[j] += 1
    # PE-diff pairs: sum of maxes = (D + sum of linear parts) / 2
    a2 = A + 0.5 * (D + float(degP @ s_m))
    ymax = B_
    stot = s_m.sum()
    term1 = 2.0 * ymax - stot - N * sy
    total = term1 / N - (2.0 * a2 - (N - 1) * stot) / (N * N)
    return np.float32(total / (B * H * W))


def kernel(predictions, targets):
    nc = build_nc()
    in_maps = _prep_inputs(predictions, targets)
    res = run_bass_kernel_spmd(nc, in_maps, list(range(N_CORES)))
    outs = [res.results[i]["o"] for i in range(N_CORES)]
    return _combine(outs, in_maps)


# revision 30
# speedup vs baseline: 1.0520x; 1.0520x over previous
"""CRPS loss kernel for Trainium2 (8 NeuronCores, batch-parallel).

Math per grid point (N=32 members x_i, target y, lat weight w_h):
  CRPS = (1/N) sum_i |x_i - y| - (1/N^2) sum_{i<j} (x_(j) - x_(i))
Everything is positively homogeneous in w_h, so the host pre-multiplies
inputs by w_h (f64) before the bf16 cast and the device computes plain
global sums.  With
  |a-b| = 2 max(a,b) - a - b
  sum_{i<j}(x_(j)-x_(i)) = 2 sum_{i<j} max(x_i,x_j) - (N-1) sum_i x_i
  max(a,b) = (a + b + |a-b|) / 2
the nonlinear device work is sums of pairwise maxes / absolute
differences; all linear sums go to the host in f64.

Layout per core: points (2 batches x 121 lat x 240 lon = 58,080, padded
to 128*454) on the partition axis, 33 "members" (32 ensemble + target)
in a free axis: X [128, 33, 454] bf16.  The 528 member pairs (incl. 32
y-pairs) split three ways:
  - DVE tensor_max (bf16 2x mode): the 32 y-pairs as one broadcast max,
    a 17-row partial of shift 5, and shifts 6..31 as 13 uniform 27-row
    slots (shift pairs (a,b), a+b=37).  400 pair-rows total.
  - PE difference matmuls: 128 pairs (shifts 1..4 plus 10 rows of
    shift 5) as one +/-1 stationary matrix against a transposed copy
    X2 [33, points] streamed from DRAM; Act abs+accumulates the f32
    diffs from PSUM (1536-col reads).
  - Accumulation of DVE slots: PE identity-matmul chains into PSUM
    (bank A = x-pairs, bank B = y-pairs) for 12 units; Act directly
    Copy+accum_outs the first two 27-row slots to offload PE.
Output per core: [128, 5] f32 partial sums; host combines in f64.
"""

import numpy as np
import ml_dtypes

import concourse.bass as bass
import concourse.mybir as mybir
from concourse.bass_utils import run_bass_kernel_spmd

H, W, B, N = 121, 240, 16, 32
N_CORES = 8
B_LOC = B // N_CORES

PTS = B_LOC * H * W          # 58,080 real points per core
PP = 454                     # free-dim columns per partition
NPTS = 128 * PP              # 58,112 padded points
M = N + 1                    # 32 ensemble members + target

# --- pair assignment ---------------------------------------------------
# PE-diff pairs: shifts 1..4 (118 pairs) + first 10 rows of shift 5.
PE_PAIRS = [(i, i + d) for d in range(1, 5) for i in range(N - d)] + [
    (i, i + 5) for i in range(10)
]
assert len(PE_PAIRS) == 128
S5_LO, S5_ROWS = 10, 17      # shift-5 rows 10..26 on DVE
DVE_PAIRS = [(6, 31), (7, 30), (8, 29), (9, 28), (10, 27), (11, 26),
             (12, 25), (13, 24), (14, 23), (15, 22), (16, 21), (17, 20),
             (18, 19)]
SLOT_ROWS = 27               # (32-a)+(32-b) with a+b=37
N_ACT_SLOTS = 1              # first DVE slot accumulated by Act

CH = 512                     # psum chunk columns
GCOLS = 3 * CH               # 1536-col diff groups (3 banks)
NGRP = (NPTS + GCOLS - 1) // GCOLS       # 38 diff groups per iter
XCH_G = 3                    # x2 dma chunk = 3 groups
NXCH = (NGRP + XCH_G - 1) // XCH_G       # 13 x2 chunks per iter
OUT_COLS = 5                 # pA, pB, D, act0, s5 sums
SUB_ROWS = 3                 # act slot sub-accum rows (27 = 9 x 3)
SUBS = SLOT_ROWS // SUB_ROWS             # 9
SUBS5 = (S5_ROWS + SUB_ROWS - 1) // SUB_ROWS   # 6 (last is 2 rows)


F32 = mybir.dt.float32
BF16 = mybir.dt.bfloat16
ALU = mybir.AluOpType
AFT = mybir.ActivationFunctionType

_NC_CACHE = {}


def _grp_cols(g):
    lo = g * GCOLS
    return lo, min(NPTS, lo + GCOLS)


def build_nc(repeat=1, detect_races=True):
    key = (repeat, detect_races)
    if key in _NC_CACHE:
        return _NC_CACHE[key]
    nc = bass.Bass(detect_race_conditions=detect_races)
    x_in = nc.declare_dram_parameter("x", [128, M * PP], BF16, isOutput=False)
    x2_in = nc.declare_dram_parameter("x2", [M, NPTS], BF16, isOutput=False)
    m_in = nc.declare_dram_parameter("mpairs", [M, 128], BF16, isOutput=False)
    i_in = nc.declare_dram_parameter("ident", [128, 128], BF16, isOutput=False)
    o_out = nc.declare_dram_parameter("o", [128, OUT_COLS], F32, isOutput=True)

    SLOT_FLAT = SLOT_ROWS * PP           # 12,258
    Y_FLAT = N * PP                      # 14,528
    S5_FLAT = S5_ROWS * PP               # 7,718
    NCH_S = (SLOT_FLAT + CH - 1) // CH   # 24
    NCH_Y = (Y_FLAT + CH - 1) // CH      # 29
    NCH_5 = (S5_FLAT + CH - 1) // CH     # 16
    NSLOT = len(DVE_PAIRS)               # 13
    XCOLS = XCH_G * GCOLS                # 4608 cols per x2 chunk

    # ---------- static schedules ----------
    # DVE production order (unit name, ring buffer id)
    dve_order = ["act0", "y", "s5"] + [f"slot{k}" for k in range(1, NSLOT)]
    # two v_sem incs per ring-slot unit (after each of its two maxes), one
    # for y/s5; v_pos = value when unit fully done, v_pos_a = first half
    v_pos, v_pos_a = {}, {}
    _v = 0
    for u in dve_order:
        if u == "y" or u == "s5":
            _v += 1
        else:
            v_pos_a[u] = _v + 1
            _v += 2
        v_pos[u] = _v
    VPER = _v                            # 28

    # PE order: interleave diff work (by x2 chunk) with accumulation so PE
    # has work before DVE's first PE-slot lands and chunks stay 2-ring.
    pe_order = []
    pe_order += [("chunk", 0), ("chunk", 1)]
    pe_order += [("y", None), ("chunk", 2)]
    ci = 3
    for k in range(1, NSLOT):
        pe_order.append(("slot", k))
        if ci < NXCH - 1 and k <= 9:
            pe_order.append(("chunk", ci))
            ci += 1
    while ci < NXCH:
        pe_order.append(("chunk", ci))
        ci += 1
    # p_sem increment schedule: +1 per diff GROUP, per y/s5/slot unit.
    p_pos = {}       # unit -> p_sem value when its last inc fires
    g_pos = {}       # diff group -> p_sem value when done
    cnt = 0
    for kind, arg in pe_order:
        if kind == "chunk":
            for g in range(arg * XCH_G, min(NGRP, (arg + 1) * XCH_G)):
                cnt += 1
                g_pos[g] = cnt
            p_pos[("chunk", arg)] = cnt
        else:
            cnt += 1
            p_pos[(kind, arg)] = cnt
    PPER = cnt                           # p_sem incs per iter (38+13=51)

    from contextlib import ExitStack

    with ExitStack() as ctx:
        xt = ctx.enter_context(nc.sbuf_tensor([128, M, PP], BF16))
        x2t = ctx.enter_context(nc.sbuf_tensor([M, 2, XCOLS], BF16))
        mp = ctx.enter_context(nc.sbuf_tensor([M, 128], BF16))
        ident = ctx.enter_context(nc.sbuf_tensor([128, 128], BF16))
        sa = ctx.enter_context(nc.sbuf_tensor([128, SLOT_ROWS, PP], BF16))
        sb = ctx.enter_context(nc.sbuf_tensor([128, SLOT_ROWS, PP], BF16))
        sc = ctx.enter_context(nc.sbuf_tensor([128, SLOT_ROWS, PP], BF16))
        sact = ctx.enter_context(nc.sbuf_tensor([128, SLOT_ROWS, PP], BF16))
        ys = ctx.enter_context(nc.sbuf_tensor([128, N, PP], BF16))
        s5t = ctx.enter_context(nc.sbuf_tensor([128, S5_ROWS, PP], BF16))
        dump_g0 = ctx.enter_context(nc.sbuf_tensor([128, GCOLS], BF16))
        dump_g1 = ctx.enter_context(nc.sbuf_tensor([128, GCOLS], BF16))
        acc_d = ctx.enter_context(nc.sbuf_tensor([128, NGRP], F32))
        acc_a = ctx.enter_context(nc.sbuf_tensor([128, SUBS], F32))
        fd_d = ctx.enter_context(nc.sbuf_tensor([128, NGRP], F32))
        fd_a = ctx.enter_context(nc.sbuf_tensor([128, SUBS], F32))
        fd_s5 = ctx.enter_context(nc.sbuf_tensor([128, SUBS5], F32))
        acc_s5 = ctx.enter_context(nc.sbuf_tensor([128, SUBS5], F32))
        ot = ctx.enter_context(nc.sbuf_tensor([128, OUT_COLS], F32))
        pA = ctx.enter_context(nc.psum_tensor([128, CH], F32))
        pB = ctx.enter_context(nc.psum_tensor([128, CH], F32))
        pd0 = ctx.enter_context(nc.psum_tensor([128, GCOLS], F32))
        pd1 = ctx.enter_context(nc.psum_tensor([128, GCOLS], F32))
        dma_sem = ctx.enter_context(nc.semaphore())
        x2_sem = ctx.enter_context(nc.semaphore())  # x2 chunk dma arrivals
        v_sem = ctx.enter_context(nc.semaphore())
        p_sem = ctx.enter_context(nc.semaphore())
        r_sem = ctx.enter_context(nc.semaphore())   # ACT diff-reads
        a_sem = ctx.enter_context(nc.semaphore())   # ACT act0-slot accums
        s5_sem = ctx.enter_context(nc.semaphore())  # ACT s5 accums
        f_sem = ctx.enter_context(nc.semaphore())   # ACT pA/pB finals done
        s_sem = ctx.enter_context(nc.semaphore())   # ACT finals
        block = ctx.enter_context(nc.Block())
        xv = xt[:]
        ring = [sa, sb, sc]
        ring_flat = [sa[:].rearrange("p r w -> p (r w)"),
                     sb[:].rearrange("p r w -> p (r w)"),
                     sc[:].rearrange("p r w -> p (r w)")]
        ys_flat = ys[:].rearrange("p r w -> p (r w)")
        s5_flat = s5t[:].rearrange("p r w -> p (r w)")
        pd = [pd0, pd1]

        @block.sync
        def _(sync):
            sync.dma_start(
                out=xt[:],
                in_=x_in[:].rearrange("p (m w) -> p m w", m=M, w=PP),
            ).then_inc(dma_sem, 16)
            sync.dma_start(out=ident[:], in_=i_in[:]).then_inc(dma_sem, 16)
            sync.dma_start(out=mp[:], in_=m_in[:]).then_inc(dma_sem, 16)
            for it in range(repeat):
                for c in range(NXCH):
                    ci_abs = it * NXCH + c
                    if ci_abs >= 1:
                        # anchor chunk completion order for the race detector
                        sync.wait_ge(x2_sem, 16 * ci_abs)
                    if ci_abs >= 2:
                        # buffer reuse: PE done with chunk ci_abs-2
                        pit, pc = divmod(ci_abs - 2, NXCH)
                        sync.wait_ge(p_sem, pit * PPER + p_pos[("chunk", pc)])
                    lo = c * XCOLS
                    hi = min(NPTS, lo + XCOLS)
                    sync.dma_start(
                        out=x2t[:, ci_abs % 2, : hi - lo],
                        in_=x2_in[:, lo:hi],
                    ).then_inc(x2_sem, 16)
            sync.wait_ge(s_sem, repeat)
            sync.dma_start(out=o_out[:], in_=ot[:]).then_inc(dma_sem, 16)

        @block.vector
        def _(vector):
            vector.wait_ge(dma_sem, 48)
            ybc = xt[:, N : N + 1, :].broadcast_to((128, N, PP))
            for it in range(repeat):
                vb = VPER * it
                for ui, unit in enumerate(dve_order):
                    # Ring users in order: act0 (=k 0), slot1..slot12; the
                    # buffer of user u is reused by user u+3.  act0 is
                    # consumed by ACT (a_sem); PE slots by p_sem.
                    if unit == "y":
                        if it > 0:
                            vector.wait_ge(p_sem, (it - 1) * PPER + p_pos[("y", None)])
                        nc.vector.tensor_max(
                            ys[:], xt[:, 0:N, :], ybc
                        ).then_inc(v_sem, 1)
                        continue
                    if unit == "s5":
                        if it > 0:
                            vector.wait_ge(s5_sem, it)  # ACT s5-accum done
                        nc.vector.tensor_max(
                            s5t[:],
                            xv[:, S5_LO + 5 : S5_LO + 5 + S5_ROWS, :],
                            xv[:, S5_LO : S5_LO + S5_ROWS, :],
                        ).then_inc(v_sem, 1)
                        continue
                    if unit == "act0":
                        if it > 0:
                            vector.wait_ge(a_sem, it)  # ACT act0-accum done
                        da, db = DVE_PAIRS[0]
                        ra = N - da
                        nc.vector.tensor_max(
                            sact[:, :ra, :], xv[:, da:N, :], xv[:, : N - da, :]
                        ).then_inc(v_sem, 1)
                        nc.vector.tensor_max(
                            sact[:, ra:SLOT_ROWS, :],
                            xv[:, db:N, :],
                            xv[:, : N - db, :],
                        ).then_inc(v_sem, 1)
                        continue
                    k = int(unit[4:])
                    # ring[(k-1) % 3]: previous occupant is slot k-3, or
                    # slot k+9 of the previous iteration for k in {1,2,3}
                    if k >= 4:
                        vector.wait_ge(p_sem, it * PPER + p_pos[("slot", k - 3)])
                    elif it > 0:
                        vector.wait_ge(
                            p_sem, (it - 1) * PPER + p_pos[("slot", k + 9)]
                        )
                    slot = ring[(k - 1) % 3]
                    da, db = DVE_PAIRS[k]
                    ra = N - da
                    nc.vector.tensor_max(
                        slot[:, :ra, :], xv[:, da:N, :], xv[:, : N - da, :]
                    ).then_inc(v_sem, 1)
                    nc.vector.tensor_max(
                        slot[:, ra:SLOT_ROWS, :],
                        xv[:, db:N, :],
                        xv[:, : N - db, :],
                    ).then_inc(v_sem, 1)

        @block.tensor
        def _(tensor):
            tensor.wait_ge(dma_sem, 48)

            def accum_chain(psum, flat, nch, flat_len, start, stop):
                last = None
                for c in range(nch):
                    lo = c * CH
                    hi = min(flat_len, lo + CH)
                    last = tensor.matmul(
                        psum[:, : hi - lo],
                        ident[:],
                        flat[:, lo:hi],
                        start=(start and c == 0),
                        stop=(stop and c == nch - 1),
                    )
                return last

            for it in range(repeat):
                vb = VPER * it
                if it > 0:
                    tensor.wait_ge(f_sem, it)  # ACT read pA/pB of prev iter
                for kind, arg in pe_order:
                    if kind == "chunk":
                        c = arg
                        ci_abs = it * NXCH + c
                        tensor.wait_ge(x2_sem, 16 * (ci_abs + 1))
                        xbuf = x2t[:, ci_abs % 2, :]
                        for g in range(c * XCH_G, min(NGRP, (c + 1) * XCH_G)):
                            gi_abs = it * NGRP + g
                            if gi_abs >= 2:
                                tensor.wait_ge(r_sem, gi_abs - 1)
                            lo, hi = _grp_cols(g)
                            xlo = lo - c * XCOLS
                            last = None
                            for s in range(0, hi - lo, CH):
                                n = min(CH, hi - lo - s)
                                last = tensor.matmul(
                                    pd[gi_abs % 2][:, s : s + n],
                                    mp[:],
                                    xbuf[:, xlo + s : xlo + s + n],
                                    start=True,
                                    stop=True,
                                )
                            last.then_inc(p_sem, 1)
                    elif kind == "y":
                        tensor.wait_ge(v_sem, vb + v_pos["y"])
                        accum_chain(pB, ys_flat, NCH_Y, Y_FLAT, True, True).then_inc(
                            p_sem, 1
                        )
                    else:  # slot k
                        k = arg
                        ra = N - DVE_PAIRS[k][0]
                        waited_b = False
                        tensor.wait_ge(v_sem, vb + v_pos_a[f"slot{k}"])
                        last = None
                        for c in range(NCH_S):
                            lo = c * CH
                            hi = min(SLOT_FLAT, lo + CH)
                            if not waited_b and (hi - 1) // PP >= ra:
                                tensor.wait_ge(v_sem, vb + v_pos[f"slot{k}"])
                                waited_b = True
                            last = tensor.matmul(
                                pA[:, : hi - lo],
                                ident[:],
                                ring_flat[(k - 1) % 3][:, lo:hi],
                                start=(k == 1 and c == 0),
                                stop=(k == NSLOT - 1 and c == NCH_S - 1),
                            )
                        last.then_inc(p_sem, 1)

        @block.scalar
        def _(scalar):
            # Per-iteration ACT order: diff reads with act0/s5 sub-accums
            # interleaved (pauses <= ~1.5us each so the 2-deep PE<->ACT psum
            # ring never starves), pA/pB finals slotted before the last two
            # reads (they only need slot12, freeing next iter's PE via
            # f_sem), then the accumulator roll-ups.
            ops = []
            gi = 0
            for s in range(SUBS):            # act0: 9 subs against reads 0-8
                ops.append(("read", gi)); gi += 1
                ops.append(("suba", s))
            for s in range(SUBS5):           # s5: 6 subs against reads 9-14
                ops.append(("read", gi)); gi += 1
                ops.append(("sub5", s))
            while gi < NGRP - 2:
                ops.append(("read", gi)); gi += 1
            ops.append(("finAB", None))
            while gi < NGRP:
                ops.append(("read", gi)); gi += 1
            ops.append(("finD", None))

            dump_g = [dump_g0, dump_g1]
            for it in range(repeat):
                vb = VPER * it
                for kind, arg in ops:
                    if kind == "read":
                        g = arg
                        gi_abs = it * NGRP + g
                        if g == 0 and it > 0:
                            scalar.wait_ge(s_sem, it)  # prev finD read acc_d
                        scalar.wait_ge(p_sem, it * PPER + g_pos[g])
                        lo, hi = _grp_cols(g)
                        nc.scalar.activation(
                            dump_g[g % 2][:, : hi - lo],
                            pd[gi_abs % 2][:, : hi - lo],
                            AFT.Abs,
                            accum_out=acc_d[:, g : g + 1],
                        ).then_inc(r_sem, 1)
                    elif kind == "suba":
                        s = arg
                        ra0 = N - DVE_PAIRS[0][0]
                        if s == 0:
                            scalar.wait_ge(v_sem, vb + v_pos_a["act0"])
                        if s * SUB_ROWS + SUB_ROWS > ra0 and (s - 1) * SUB_ROWS + SUB_ROWS <= ra0:
                            scalar.wait_ge(v_sem, vb + v_pos["act0"])
                        r0 = s * SUB_ROWS
                        sl = sact[:, r0 : r0 + SUB_ROWS, :].rearrange(
                            "p r w -> p (r w)"
                        )
                        act = nc.scalar.activation(
                            sl, sl, AFT.Copy, accum_out=acc_a[:, s : s + 1]
                        )
                        if s == SUBS - 1:
                            act.then_inc(a_sem, 1)
                    elif kind == "sub5":
                        s = arg
                        if s == 0:
                            scalar.wait_ge(v_sem, vb + v_pos["s5"])
                        r0 = s * SUB_ROWS
                        rows = min(SUB_ROWS, S5_ROWS - r0)
                        sl = s5t[:, r0 : r0 + rows, :].rearrange("p r w -> p (r w)")
                        act = nc.scalar.activation(
                            sl, sl, AFT.Copy, accum_out=acc_s5[:, s : s + 1]
                        )
                        if s == SUBS5 - 1:
                            act.then_inc(s5_sem, 1)
                    elif kind == "finAB":
                        scalar.wait_ge(p_sem, it * PPER + p_pos[("slot", NSLOT - 1)])
                        nc.scalar.activation(
                            pA[:], pA[:], AFT.Copy, accum_out=ot[:, 0:1]
                        )
                        nc.scalar.activation(
                            pB[:], pB[:], AFT.Copy, accum_out=ot[:, 1:2]
                        ).then_inc(f_sem, 1)
                    else:  # finD
                        scalar.wait_ge(r_sem, (it + 1) * NGRP)
                        scalar.wait_ge(a_sem, it + 1)
                        scalar.wait_ge(s5_sem, it + 1)
                        nc.scalar.activation(
                            fd_d[:], acc_d[:], AFT.Copy, accum_out=ot[:, 2:3]
                        )
                        nc.scalar.activation(
                            fd_a[:], acc_a[:], AFT.Copy, accum_out=ot[:, 3:4]
                        )
                        nc.scalar.activation(
                            fd_s5[:], acc_s5[:], AFT.Copy, accum_out=ot[:, 4:5]
                        ).then_inc(s_sem, 1)

    _NC_CACHE[key] = nc
    return nc


def _lat_weights_f64():
    lats = np.arange(90.0, -91.5, -1.5)  # [121]
    w = np.cos(np.deg2rad(lats))
    return H * (w / np.sum(w))


def _prep_inputs(predictions, targets):
    """Full f32 -> per-core packed bf16 maps (pre-weighted, padded)."""
    w = _lat_weights_f64()
    p = np.asarray(predictions, dtype=np.float64) * w[None, None, :, None]
    t = np.asarray(targets, dtype=np.float64) * w[None, :, None]
    ident = np.eye(128).astype(ml_dtypes.bfloat16)
    mpairs = np.zeros((M, 128), ml_dtypes.bfloat16)
    for c, (i, j) in enumerate(PE_PAIRS):
        mpairs[i, c] = 1
        mpairs[j, c] = -1
    in_maps = []
    for c in range(N_CORES):
        pc = p[B_LOC * c : B_LOC * (c + 1)]          # [2,32,121,240]
        tc = t[B_LOC * c : B_LOC * (c + 1)]          # [2,121,240]
        arr = np.zeros((NPTS, M), np.float64)
        arr[:PTS, :N] = pc.transpose(0, 2, 3, 1).reshape(PTS, N)
        arr[:PTS, N] = tc.reshape(PTS)
        xw = arr.astype(ml_dtypes.bfloat16)          # [58112, 33]
        xc = np.ascontiguousarray(
            xw.reshape(128, PP, M).transpose(0, 2, 1)
        ).reshape(128, M * PP)
        x2 = np.ascontiguousarray(xw.T)              # [33, 58112]
        in_maps.append({"x": xc, "x2": x2, "mpairs": mpairs, "ident": ident})
    return in_maps


def _member_sums(in_maps):
    """f64 per-member weighted sums from the exact bf16 device values."""
    s_m = np.zeros(N, np.float64)
    sy = 0.0
    for m in in_maps:
        x2 = m["x2"].astype(np.float64)              # [33, NPTS]
        s_m += x2[:N].sum(axis=1)
        sy += x2[N].sum()
    return s_m, sy


def _combine(outs, in_maps):
    """outs: list of [128, OUT_COLS] f32 -> scalar f32 (host math in f64)."""
    s_m, sy = _member_sums(in_maps)
    A = B_ = D = 0.0
    for o in outs:
        o = np.asarray(o, dtype=np.float64)
        A += o[:, 0].sum() + o[:, 3].sum() + o[:, 4].sum()
        B_ += o[:, 1].sum()
        D += o[:, 2].sum()
    degP = np.zeros(N, np.float64)
    for (i, j) in PE_PAIRS:
        degP[i] += 1
        degP[j] += 1
    # PE-diff pairs: sum of maxes = (D + sum of linear parts) / 2
    a2 = A + 0.5 * (D + float(degP @ s_m))
    ymax = B_
    stot = s_m.sum()
    term1 = 2.0 * ymax - stot - N * sy
    total = term1 / N - (2.0 * a2 - (N - 1) * stot) / (N * N)
    return np.float32(total / (B * H * W))


def kernel(predictions, targets):
    nc = build_nc()
    in_maps = _prep_inputs(predictions, targets)
    res = run_bass_kernel_spmd(nc, in_maps, list(range(N_CORES)))
    outs = [res.results[i]["o"] for i in range(N_CORES)]
    return _combine(outs, in_maps)
